# revision 24
# baseline (speedup 1.0000x reference)
"""Trainium2 Bass kernel for nn_Decoder (gnn_message_passing).

12-step LSTM decoder with (N,N) pairwise pooling, N=512 agents, sharded over
8 NeuronCores by agent rows (64 agents/core).

Key algebra: the pairwise MLP first layer collapses:
  feat[i,j] = [corr@W_se | h[j] | h[i]],  corr[i,j] = pos[i]-pos[j]
  feat @ W1 + b1 = P[i] + Q[j]
  P[i] = pos[i]@A + h[i]@W1[40:48] + (b1 + b_se@W1[0:32])
  Q[j] = h[j]@W1[32:40] - pos[j]@A,   A = W_se @ W1[0:32]   (64-dim)
so per step each core computes its own P (64 agents) and Q-block, all-gathers
Q (the only cross-core exchange), then for each pair (i,j):
  ph = relu( relu(P[i]+Q[j]) @ W2 + b2 );  ctx[i] = masked-max_j ph
The masked max folds the neighbor mask into the PE via an accumulated
"mask matmul" adding BIG*nei[i,j] to PSUM, then a plain reduce_max and a
final relu(x - BIG + b2) epilogue.

Pooling layout per core/step: agents il = 16b + 8q + 4p + t
  (t = PSUM tile 0..3, b = 32-partition strip 0..3, q = strip half, p = parity)
PSUM tile t partition u = 32b + 16q + 2f + p  (f = feature 0..7), dense via
two zero-padded M=32 matmuls accumulated per strip.

Latency optimizations over the v1 kernel:
- ctx never transposed to [8, NA]: the reduce result stays in partition
  layout; R[u, il] = relu(ctx_mx+bias)[u] * M2[u, il] (one broadcast mult),
  and all ctx consumers (W_in / W_m / W_v ctx parts) use E8-expanded
  [128, *] weights so the matmul contraction picks the matching f lane.
  Kills the DRAM round-trip + 5 DMAs per step.
- pdup built on the PE (4 matmuls with agent-permuted rhs + 1 bias copy)
  instead of 4 SBUF-SBUF DMAs.
- LSTM gate psum packed [32, NA] (g|i|f|o) so one Tanh + one Sigmoid
  activation cover all gates.
- exp(0.5*lv) = sig/(sig-1) with host-negated eps (one sigmoid + vector
  sub/div) instead of two sigmoids + reciprocal + extra mult.
- Whh@h and Wm/Wv@h matmuls for step s+1/s issued before the pooling
  matmuls so the in-order PE queue runs them inside the exchange window.
- pooling h1 builds split vector/scalar (24/8; the gpsimd Q7 runs
  tensor_scalar at ~7.5us per [128,512] tile - never give it builds);
  qdup split across the two HWDGE rings.
"""
import numpy as np
from contextlib import ExitStack

import concourse.bass as bass
import concourse.bacc as bacc
import concourse.mybir as mybir
from concourse import tile
from concourse.bass_utils import run_bass_kernel_spmd

F32 = mybir.dt.float32
BF16 = mybir.dt.bfloat16
F8 = mybir.dt.float8e4
I32 = mybir.dt.int32


class _View:
    """Lets blob sub-APs be indexed like tiles: v[:, :] / v[:, a:b]."""
    def __init__(self, ap):
        self.ap = ap

    def __getitem__(self, key):
        return self.ap[key]

N = 512
R = 8            # cores
NA = N // R      # agents per core = 64
NSTEPS = 12
D = 64           # pooling hidden dim
BIG = 512.0

AluOp = mybir.AluOpType
Act = mybir.ActivationFunctionType


# ---------------------------------------------------------------------------
# host-side constant packing
# ---------------------------------------------------------------------------

def _f_of_u():
    # u = 32b + 16q + 2f + p -> f
    u = np.arange(128)
    return (u >> 1) & 7


def build_constants(W_in, b_in, W_ih, W_hh, b_ih, b_hh, W_m, b_m, W_v, b_v,
                    W_zh, b_zh, W_se, b_se, W1, b1, W2, b2):
    c = {}
    A = W_se @ W1[0:32]                      # [2, 64]
    c["A_T"] = np.ascontiguousarray(A)       # lhsT [2, 64] for +a
    c["negA_T"] = np.ascontiguousarray(-A)
    c["W1u_T"] = np.ascontiguousarray(W1[32:40])   # [8, 64] lhsT for u (hj)
    c["W1v_T"] = np.ascontiguousarray(W1[40:48])   # [8, 64] lhsT for v (hi)
    b1p = (b1 + b_se @ W1[0:32]).astype(np.float32)          # [64]
    c["b1p_dup"] = np.tile(b1p, 2).reshape(128, 1)           # both halves

    # pooling lhsT per strip-half q: [128, 32], col m = 16q + 2f + p
    for q in range(2):
        L = np.zeros((128, 32), dtype=np.float32)
        for p in range(2):
            for f in range(8):
                L[p * 64:(p + 1) * 64, 16 * q + 2 * f + p] = W2[:, f]
        c[f"Wpool_q{q}"] = L

    # mask lhsT [16, 128]: k = 4b + 2q + p -> BIG at u = 32b + 16q + 2f + p
    LM = np.zeros((16, 128), dtype=np.float32)
    for p in range(2):
        for b in range(4):
            for q in range(2):
                k = 4 * b + 2 * q + p
                for f in range(8):
                    LM[k, 32 * b + 16 * q + 2 * f + p] = BIG
    c["lhsT_mask"] = LM

    # ctx epilogue bias [128, 1]: b2[f] - BIG at u
    fu = _f_of_u()
    c["bias_ctx"] = (b2[fu] - BIG).reshape(128, 1).astype(np.float32)

    # M2[u, il] = 1 iff agent-bits of u match il (il = 16b + 8q + 4p + t)
    M2 = np.zeros((128, NA), dtype=np.float32)
    u = np.arange(128)
    ub, uq, up = (u >> 5) & 3, (u >> 4) & 1, u & 1
    il = np.arange(NA)
    ib, iq, ip = (il >> 4) & 3, (il >> 3) & 1, (il >> 2) & 1
    M2[:, :] = ((ub[:, None] == ib[None, :]) & (uq[:, None] == iq[None, :])
                & (up[:, None] == ip[None, :])).astype(np.float32)
    c["M2"] = M2

    # x layer: x = relu(concat([ctx, prev, c, z]) @ W_in + b_in)
    # ctx part expanded to read R[u, il] directly
    c["Winx_big"] = np.ascontiguousarray(W_in[0:8][fu])   # [128, 16]
    c["Win_prev"] = np.ascontiguousarray(W_in[8:10])      # [2, 16]
    c["Win_c"] = np.ascontiguousarray(W_in[10:18])        # [8, 16]
    c["Win_z"] = np.ascontiguousarray(W_in[18:20])        # [2, 16]
    c["b_in"] = b_in.reshape(16, 1).astype(np.float32)

    # gates (torch order i,f,g,o in columns of W_ih/W_hh); psum layout
    # [8, 4*NA]: free-dim blocks g|i|f|o so one Tanh + one Sigmoid cover all
    # gates and every consumer slices the free dim (engine partition bases
    # must be 32-aligned, free offsets are unrestricted). The per-gate bias
    # rides in a 17th ones-row of xT via row 16 of each Wih lhsT.
    for gi, g in ((2, "g"), (0, "i"), (1, "f"), (3, "o")):
        sl = slice(8 * gi, 8 * gi + 8)
        wih = np.zeros((33, 8), dtype=np.float32)
        wih[0:16] = W_ih[:, sl]
        wih[32] = b_ih[sl] + b_hh[sl]
        c[f"Wih_{g}"] = wih
        c[f"Whh_{g}"] = np.ascontiguousarray(W_hh[:, sl])

    # mu/logvar: mu = [h[:, :4], ctx] @ W_m + b_m ; lv = [h[:, 4:], ctx] @ W_v
    Wmh = np.zeros((8, 2), dtype=np.float32); Wmh[0:4] = W_m[0:4]
    Wlh = np.zeros((8, 2), dtype=np.float32); Wlh[4:8] = W_v[0:4]
    c["Wm_h"] = Wmh
    c["Wv_h"] = Wlh
    c["Wm_big"] = np.ascontiguousarray(W_m[4:12][fu])   # [128, 2]
    c["Wv_big"] = np.ascontiguousarray(W_v[4:12][fu])   # [128, 2]
    c["b_m"] = b_m.reshape(2, 1).astype(np.float32)
    c["b_v"] = b_v.reshape(2, 1).astype(np.float32)
    c["half_b_v"] = (0.5 * b_v).reshape(2, 1).astype(np.float32)

    c["Wzh_T"] = np.ascontiguousarray(W_zh)         # [2, 8]
    c["b_zh"] = b_zh.reshape(8, 1).astype(np.float32)
    return c


BF16_CONSTS = {"Wpool_q0", "Wpool_q1", "lhsT_mask",
               "Wih_g", "Wih_i", "Wih_f", "Wih_o",
               "Winx_big", "Wm_big", "Wv_big", "M2"}

CONST_SHAPES = {
    "A_T": [2, D], "negA_T": [2, D], "W1u_T": [8, D], "W1v_T": [8, D],
    "b1p_dup": [128, 1], "Wpool_q0": [128, 32], "Wpool_q1": [128, 32],
    "lhsT_mask": [16, 128], "bias_ctx": [128, 1], "M2": [128, NA],
    "Winx_big": [128, 16], "Win_prev": [2, 16], "Win_c": [8, 16],
    "Win_z": [2, 16], "b_in": [16, 1],
    "Wih_g": [33, 8], "Whh_g": [8, 8], "Wih_i": [33, 8], "Whh_i": [8, 8],
    "Wih_f": [33, 8], "Whh_f": [8, 8], "Wih_o": [33, 8], "Whh_o": [8, 8],
    "Wm_h": [8, 2], "Wv_h": [8, 2], "Wm_big": [128, 2], "Wv_big": [128, 2],
    "b_m": [2, 1], "b_v": [2, 1], "half_b_v": [2, 1],
    "Wzh_T": [2, 8], "b_zh": [8, 1],
}


def _blob_layout():
    offs_f, offs_b = {}, {}
    cf = cb = 0
    for name, (p, w) in CONST_SHAPES.items():
        if name in BF16_CONSTS:
            offs_b[name] = (p, cb, w)
            cb += w
        else:
            offs_f[name] = (p, cf, w)
            cf += w
    return offs_f, cf, offs_b, cb


OFFS_F, CF, OFFS_B, CB = _blob_layout()
# per-core input blob [8, CP]: name -> (rows, col, cols)
PER_CORE = {"cT": (8, 0, NA), "c0T": (8, NA, NA), "pT": (2, 2 * NA, NA),
            "zT": (2, 3 * NA, NA), "obslastT": (2, 4 * NA, NA),
            "epsT": (2, 5 * NA, NSTEPS * NA)}
CP = 5 * NA + NSTEPS * NA


# ---------------------------------------------------------------------------
# device program
# ---------------------------------------------------------------------------

def build_program(nsteps=NSTEPS, debug=False):
    nc = bacc.Bacc("TRN2", target_bir_lowering=False, debug=False,
                   num_devices=R)

    io = {}
    io["nei_own"] = nc.dram_tensor("nei_own", [nsteps, NA, N], I32, kind="ExternalInput")
    io["cblob_f"] = nc.dram_tensor("cblob_f", [128, CF], F32, kind="ExternalInput")
    io["cblob_b"] = nc.dram_tensor("cblob_b", [128, CB], BF16, kind="ExternalInput")
    io["pblob"] = nc.dram_tensor("pblob", [8, CP], F32, kind="ExternalInput")

    # outputs [2, nsteps, NA]
    o_pos = nc.dram_tensor("out_positions", [2, nsteps, NA], F32, kind="ExternalOutput")
    o_mu = nc.dram_tensor("out_means", [2, nsteps, NA], F32, kind="ExternalOutput")
    o_lv = nc.dram_tensor("out_logvars", [2, nsteps, NA], F32, kind="ExternalOutput")
    dbg = {}
    if debug:
        for nm, shp in (("d_x", [33, NA]), ("d_h", [8, NA]),
                        ("d_pdup", [128, 32]), ("d_qdup", [128, N]),
                        ("d_ctxmx", [128, 4]), ("d_R", [128, NA]),
                        ("d_tang", [8, NA]), ("d_sifo", [8, 3 * NA]),
                        ("d_cl", [8, NA])):
            dbg[nm] = nc.dram_tensor(nm, shp, F32, kind="ExternalOutput")

    with tile.TileContext(nc) as tc, ExitStack() as ctx:
        sb1 = ctx.enter_context(tc.tile_pool(name="consts", bufs=1))
        sbs = ctx.enter_context(tc.tile_pool(name="state", bufs=2))
        sbw = ctx.enter_context(tc.tile_pool(name="work", bufs=3))
        sbh = ctx.enter_context(tc.tile_pool(name="h1p", bufs=4))
        sbm = ctx.enter_context(tc.tile_pool(name="maskp", bufs=8))
        pp = ctx.enter_context(tc.tile_pool(name="poolps", bufs=4, space="PSUM"))
        sp = ctx.enter_context(tc.tile_pool(name="smallps", bufs=1, space="PSUM"))
        spg = ctx.enter_context(tc.tile_pool(name="gateps", bufs=1, space="PSUM"))
        dr = ctx.enter_context(tc.tile_pool(name="dram", bufs=2, space="DRAM"))

        # warm-up collective: the first collective call pays ~40us of ncfw
        # staging + inter-core startup skew; firing a dummy AllGather first
        # overlaps that cost with constant loading and the step-0 LSTM
        warm_in = dr.tile([D, NA], BF16, tag="warm_in")
        warm_out = dr.tile([R * D, NA], BF16, tag="warm_out")
        nc.gpsimd.collective_compute(
            "AllGather", AluOp.bypass, replica_groups=[list(range(R))],
            ins=[warm_in[:, :]], outs=[warm_out[:, :]])

        # ---- load constants / inputs via 3 blob DMAs ----
        tf = sb1.tile([128, CF], F32, tag="blob_f")
        nc.sync.dma_start(tf[:, :], io["cblob_f"][:, :])
        tb = sb1.tile([128, CB], BF16, tag="blob_b")
        nc.sync.dma_start(tb[:, :], io["cblob_b"][:, :])
        tp_ = sb1.tile([8, CP], F32, tag="blob_p")
        nc.scalar.dma_start(tp_[:, :], io["pblob"][:, :])
        cst = {}
        for name, (p, off, w) in OFFS_F.items():
            cst[name] = _View(tf[0:p, off:off + w])
        for name, (p, off, w) in OFFS_B.items():
            cst[name] = _View(tb[0:p, off:off + w])
        pc = {name: tp_[0:p, off:off + w]
              for name, (p, off, w) in PER_CORE.items()}
        cT, zT, pT, epsT = pc["cT"], pc["zT"], pc["pT"], pc["epsT"]
        posT, clT = pc["obslastT"], pc["c0T"]

        # shared PSUM bank, packed by disjoint partition ranges (a matmul
        # start zeroes the full 2KB bank line of the partitions it writes,
        # so regions sharing partitions cannot share a bank). ps_mv lives in
        # its own bank: sharing it with ps_x would serialize the next x
        # matmul behind this step's mu/lv/position reads (whole-tile WAR).
        psA = sp.tile([128, 512], F32, tag="psA")
        psMV = sp.tile([2, 2 * NA], F32, tag="psMV")

        # h0 = z @ W_zh + b_zh
        ps_h0 = psA[32:40, 0:NA]
        nc.tensor.matmul(ps_h0, cst["Wzh_T"][:, :], zT[:, :],
                         start=True, stop=True)
        hT = sb1.tile([8, NA], F32, tag="hT")
        nc.scalar.activation(hT[:, :], ps_h0, Act.Identity,
                             bias=cst["b_zh"][:, :])

        # R = ctx in broadcast layout [128, NA]; zero for step 0
        Rt = sb1.tile([128, NA], BF16, tag="R0")
        nc.vector.memset(Rt[:, :], 0.0)

        # xT row 16 is a persistent ones-row: the gate matmuls contract it
        # against row 16 of Wih_* to add the gate biases
        xT = sb1.tile([33, NA], BF16, tag="xT")
        nc.vector.memset(xT[:, :], 0.0)
        nc.vector.memset(xT[32:33, :], 1.0)

        # output accumulators [2, nsteps*NA]
        ob_pos = sb1.tile([2, nsteps * NA], F32, tag="ob_pos")
        ob_mu = sb1.tile([2, nsteps * NA], F32, tag="ob_mu")
        ob_lv = sb1.tile([2, nsteps * NA], F32, tag="ob_lv")

        # prologue: open gate psum for step 0 with the Whh parts
        GATES = ("g", "i", "f", "o")
        ps_g = spg.tile([8, 4 * NA], F32, tag="ps_g")
        for gi, g in enumerate(GATES):
            # one start=True per bank round: it clears has_written for all
            # partitions it writes across the full bank line, so later
            # column-block opens must use start=False (bit clear => the
            # first write overwrites anyway)
            nc.tensor.matmul(ps_g[:, gi * NA:(gi + 1) * NA],
                             cst[f"Whh_{g}"][:, :], hT[:, :],
                             start=(gi == 0), stop=False,
                             skip_group_check=True)

        # engine schedule for the 8 pooling h1 builds per tile, by 2*b + q
        BUILD_ENG = ["v", "s", "v", "v", "v", "s", "v", "s"]

        for s in range(nsteps):
            prevT = pT if s == 0 else ob_pos[:, (s - 1) * NA: s * NA]

            # ---------------- BLOCK A: masks, x, LSTM, Q, exchange ------
            # mask DMAs first on the gpsimd queue (it blocks on the
            # collective, so these must precede the trigger)
            mks = []
            for t in range(4):
                mk = sbm.tile([16, N], BF16, tag="mask")
                nc.gpsimd.dma_start(mk[:, :], io["nei_own"][s, t::4, :])
                mks.append(mk)

            # x = relu([ctx, prev, c, z] @ W_in + b_in)   -> xT [16, NA]
            ps_x = psA[0:16, 0:NA]
            nc.tensor.matmul(ps_x, cst["Win_c"][:, :], cT[:, :],
                             start=True, stop=False, skip_group_check=True)
            nc.tensor.matmul(ps_x, cst["Win_z"][:, :], zT[:, :],
                             start=False, stop=False, skip_group_check=True)
            nc.tensor.matmul(ps_x, cst["Winx_big"][:, :], Rt[:, :],
                             start=False, stop=False, skip_group_check=True)
            nc.tensor.matmul(ps_x, cst["Win_prev"][:, :], prevT,
                             start=False, stop=True, skip_group_check=True)
            nc.scalar.activation(xT[0:16, :], ps_x, Act.Relu,
                                 bias=cst["b_in"][:, :])

            # gates psum [8, 4*NA]: free blocks g|i|f|o
            # (Whh parts were issued at the end of the previous iteration)
            for gi, g in enumerate(GATES):
                nc.tensor.matmul(ps_g[:, gi * NA:(gi + 1) * NA],
                                 cst[f"Wih_{g}"][:, :], xT[:, :],
                                 start=False, stop=True, skip_group_check=True)
            tan_g = sbw.tile([8, NA], F32, tag="tan_g")
            nc.scalar.activation(tan_g[:, :], ps_g[:, 0:NA], Act.Tanh)
            sig_ifo = sbw.tile([8, 3 * NA], F32, tag="sig_ifo")
            nc.scalar.activation(sig_ifo[:, :], ps_g[:, NA:4 * NA],
                                 Act.Sigmoid)

            # cl = sig_f*cl + sig_i*tanh(g) ; h = sig_o*tanh(cl)
            t1 = sbw.tile([8, NA], F32, tag="t1")
            nc.vector.tensor_mul(t1[:, :], sig_ifo[:, 0:NA], tan_g[:, :])
            t2 = sbw.tile([8, NA], F32, tag="t2")
            nc.vector.tensor_mul(t2[:, :], sig_ifo[:, NA:2 * NA], clT[:, :])
            clT = sbs.tile([8, NA], F32, tag="clT_s")
            nc.vector.tensor_add(clT[:, :], t1[:, :], t2[:, :])
            tcl = sbw.tile([8, NA], F32, tag="tcl")
            nc.scalar.activation(tcl[:, :], clT[:, :], Act.Tanh)
            hT = sbs.tile([8, NA], F32, tag="hT_s")
            nc.vector.tensor_mul(hT[:, :], sig_ifo[:, 2 * NA:3 * NA],
                                 tcl[:, :])

            # Q = h @ W1u - pos @ A ; pos part first (ready early)
            ps_q = psA[64:128, 0:NA]
            nc.tensor.matmul(ps_q, cst["negA_T"][:, :], posT[:, :],
                             start=True, stop=False, skip_group_check=True)
            nc.tensor.matmul(ps_q, cst["W1u_T"][:, :], hT[:, :],
                             start=False, stop=True, skip_group_check=True)
            qblk = sbw.tile([D, NA], BF16, tag="qblk")
            nc.scalar.copy(qblk[:, :], ps_q)

            ag_in = dr.tile([D, NA], BF16, tag="ag_in")
            nc.sync.dma_start(ag_in[:, :], qblk[:, :])
            ag_out = dr.tile([R * D, NA], BF16, tag="ag_out")
            nc.gpsimd.collective_compute(
                "AllGather", AluOp.bypass,
                replica_groups=[list(range(R))],
                ins=[ag_in[:, :]],
                outs=[ag_out[:, :]],
            )

            # ---------------- BLOCK B: PE prefetch during exchange ------
            # P in pdup layout via agent-permuted rhs: col pk = 8b + 4q + t,
            # partition half p. psum [128, 32].
            pos_pm = posT[:, :].rearrange("k (b q p t) -> k p b q t",
                                          b=4, q=2, p=2, t=4)
            h_pm = hT[:, :].rearrange("k (b q p t) -> k p b q t",
                                      b=4, q=2, p=2, t=4)
            ps_pd = sp.tile([128, 32], F32, tag="ps_pd")
            for half in range(2):
                psl = ps_pd[64 * half:64 * half + 64, :]
                tp = (0, 64 * half)
                nc.tensor.matmul(psl, cst["A_T"][:, :], pos_pm[:, half],
                                 start=True, stop=False, tile_position=tp,
                                 skip_group_check=True)
                nc.tensor.matmul(psl, cst["W1v_T"][:, :], h_pm[:, half],
                                 start=False, stop=True, tile_position=tp,
                                 skip_group_check=True)
            pdup = sbw.tile([128, 32], F32, tag="pdup")
            nc.scalar.activation(pdup[:, :], ps_pd[:, :], Act.Identity,
                                 bias=cst["b1p_dup"][:, :])

            # mu/lv h-parts for this step
            ps_mv = psMV[:, :]
            nc.tensor.matmul(ps_mv[:, 0:NA], cst["Wm_h"][:, :], hT[:, :],
                             start=True, stop=False, skip_group_check=True)
            nc.tensor.matmul(ps_mv[:, NA:2 * NA], cst["Wv_h"][:, :], hT[:, :],
                             start=False, stop=False, skip_group_check=True)
            # gate psum Whh parts for next step
            if s + 1 < nsteps:
                ps_g = spg.tile([8, 4 * NA], F32, tag="ps_g")
                for gi, g in enumerate(GATES):
                    nc.tensor.matmul(ps_g[:, gi * NA:(gi + 1) * NA],
                                     cst[f"Whh_{g}"][:, :], hT[:, :],
                                     start=(gi == 0), stop=False,
                                     skip_group_check=True)

            # qdup [128, 512]: partition (dup, d), free j = 64*rr + jl;
            # the two halves go down the two HWDGE rings in parallel
            qdup = sbw.tile([128, N], BF16, tag="qdup")
            for half, eng in ((0, nc.sync), (1, nc.scalar)):
                eng.dma_start(
                    qdup[half * D:(half + 1) * D, :].rearrange(
                        "d (rr jl) -> d rr jl", rr=R, jl=NA),
                    ag_out.rearrange("(rr d) jl -> d rr jl", rr=R, d=D),
                )

            # ---------------- BLOCK C: pooling ----------------
            ctx_mx = sbw.tile([128, 4], F32, tag="ctx_mx")
            for t in range(4):
                pt_ = pp.tile([128, N], F32, tag="poolps")
                # mask-MM opens the accumulation; only needs the mask DMA,
                # so the PE can run it during the AllGather window
                nc.tensor.matmul(pt_[:, :], cst["lhsT_mask"][:, :],
                                 mks[t][:, :],
                                 start=True, stop=False, skip_group_check=True)
                for b in range(4):
                    for q in range(2):
                        pk = 8 * b + 4 * q + t
                        eng = {"v": nc.vector,
                               "s": nc.scalar}[BUILD_ENG[2 * b + q]]
                        h1 = sbh.tile([128, N], BF16,
                                      tag=f"h1{BUILD_ENG[2 * b + q]}")
                        if eng is nc.scalar:
                            nc.scalar.activation(
                                h1[:, :], qdup[:, :], Act.Relu,
                                bias=pdup[:, pk:pk + 1])
                        else:
                            eng.tensor_scalar(
                                h1[:, :], qdup[:, :], pdup[:, pk:pk + 1], 0.0,
                                op0=AluOp.add, op1=AluOp.max)
                        nc.tensor.matmul(
                            pt_[32 * b:32 * b + 32, :],
                            cst[f"Wpool_q{q}"][:, :], h1[:, :],
                            start=False, stop=(b == 3 and q == 1),
                            skip_group_check=True,
                            tile_position=(0, 32 * b))
                nc.vector.tensor_reduce(
                    ctx_mx[:, t:t + 1], pt_[:, :], axis=mybir.AxisListType.X,
                    op=AluOp.max)

            # ctx epilogue: relu(max - BIG + b2) broadcast into R [128, NA]
            ctx_r = sbw.tile([128, 4], F32, tag="ctx_r")
            nc.scalar.activation(ctx_r[:, :], ctx_mx[:, :],
                                 Act.Relu, bias=cst["bias_ctx"][:, :])
            Rt = sbs.tile([128, NA], BF16, tag="R_s")
            nc.vector.tensor_tensor(
                Rt[:, :].rearrange("P (r t) -> P r t", r=16, t=4),
                ctx_r[:, :].unsqueeze(1).broadcast_to([128, 16, 4]),
                cst["M2"][:, :].rearrange("P (r t) -> P r t", r=16, t=4),
                op=AluOp.mult)

            if debug and s == 0:
                dqf = sbw.tile([128, N], F32, tag="dqf")
                nc.vector.tensor_copy(dqf[:, :], qdup[:, :])
                for nm, t_ in (("d_x", xT), ("d_h", hT), ("d_pdup", pdup),
                               ("d_qdup", dqf), ("d_ctxmx", ctx_mx),
                               ("d_R", Rt), ("d_tang", tan_g),
                               ("d_sifo", sig_ifo), ("d_cl", clT)):
                    nc.sync.dma_start(dbg[nm][:, :], t_[:, :])

            # ---------------- BLOCK D: outputs + position ---------------
            nc.tensor.matmul(ps_mv[:, 0:NA], cst["Wm_big"][:, :], Rt[:, :],
                             start=False, stop=True, skip_group_check=True)
            nc.tensor.matmul(ps_mv[:, NA:2 * NA], cst["Wv_big"][:, :],
                             Rt[:, :], start=False, stop=True,
                             skip_group_check=True)
            sl = slice(s * NA, (s + 1) * NA)
            # sig = sigmoid(0.5*lv); exp(0.5*lv) = sig / sig(-) = -sig/(sig-1)
            # the minus sign is folded into host-negated eps
            sig = sbw.tile([2, NA], F32, tag="sig")
            nc.scalar.activation(sig[:, :], ps_mv[:, NA:2 * NA], Act.Sigmoid,
                                 bias=cst["half_b_v"][:, :], scale=0.5)
            nc.scalar.activation(ob_mu[:, sl], ps_mv[:, 0:NA], Act.Identity,
                                 bias=cst["b_m"][:, :])
            nc.scalar.activation(ob_lv[:, sl], ps_mv[:, NA:2 * NA],
                                 Act.Identity, bias=cst["b_v"][:, :])
            sd = sbw.tile([2, NA], F32, tag="sd")
            nc.vector.tensor_scalar(sd[:, :], sig[:, :], 1.0, None,
                                    op0=AluOp.subtract)
            pe1 = sbw.tile([2, NA], F32, tag="pe1")
            nc.vector.tensor_mul(pe1[:, :], epsT[:, sl], sig[:, :])
            rcp = sbw.tile([2, NA], F32, tag="rcp")
            nc.vector.reciprocal_approx_fast(rcp[:, :], sd[:, :])
            pe = sbw.tile([2, NA], F32, tag="pe")
            nc.vector.tensor_mul(pe[:, :], pe1[:, :], rcp[:, :])
            # ob_pos = (psum_mu + b_m) + pe ; posT += ob_pos
            nc.vector.scalar_tensor_tensor(
                ob_pos[:, sl], ps_mv[:, 0:NA], cst["b_m"][:, 0:1], pe[:, :],
                op0=AluOp.add, op1=AluOp.add)
            posT_new = sbs.tile([2, NA], F32, tag="posT_s")
            nc.vector.tensor_add(posT_new[:, :], posT[:, :], ob_pos[:, sl])
            posT = posT_new

        # final output DMAs: [2, (s, il)] -> dram [2, s, il]
        for ob, od in ((ob_pos, o_pos), (ob_mu, o_mu), (ob_lv, o_lv)):
            nc.sync.dma_start(
                od.rearrange("k s il -> k s il"),
                ob.rearrange("k (s il) -> k s il", s=nsteps, il=NA),
            )

    nc.compile()
    return nc


# ---------------------------------------------------------------------------
# host wrapper
# ---------------------------------------------------------------------------

def make_in_maps(inputs, nsteps=NSTEPS):
    inp = {k: np.asarray(v) for k, v in inputs.items()}
    cst = build_constants(
        inp["W_in"], inp["b_in"], inp["W_ih"], inp["W_hh"], inp["b_ih"],
        inp["b_hh"], inp["W_m"], inp["b_m"], inp["W_v"], inp["b_v"],
        inp["W_zh"], inp["b_zh"], inp["W_se"], inp["b_se"], inp["W1"],
        inp["b1"], inp["W2"], inp["b2"])

    blob_f = np.zeros((128, CF), dtype=np.float32)
    for name, (p, off, w) in OFFS_F.items():
        blob_f[0:p, off:off + w] = cst[name]
    import ml_dtypes
    blob_b = np.zeros((128, CB), dtype=np.float32)
    for name, (p, off, w) in OFFS_B.items():
        blob_b[0:p, off:off + w] = cst[name]
    blob_b = blob_b.astype(ml_dtypes.bfloat16)

    in_maps = []
    for r in range(R):
        sl = slice(r * NA, (r + 1) * NA)
        per = {
            "cT": inp["c"][sl].T, "c0T": inp["c0_noise"][sl].T,
            "pT": inp["p"][sl].T, "zT": inp["z"][sl].T,
            "obslastT": inp["obs_traj_pos"][-1, sl].T,
            # negated: pos = mu + eps*exp(0.5lv) done as mu + (-eps)*(-exp)
            "epsT": -inp["eps"][:nsteps, sl, :].transpose(2, 0, 1).reshape(
                2, nsteps * NA),
        }
        blob_p = np.zeros((8, CP), dtype=np.float32)
        for name, (p, off, w) in PER_CORE.items():
            blob_p[0:p, off:off + w] = per[name]
        m = {"cblob_f": blob_f, "cblob_b": blob_b, "pblob": blob_p,
             "nei_own": np.ascontiguousarray(inp["nei_index"][:nsteps, sl, :])}
        in_maps.append(m)
    return in_maps


_cached = {}


def kernel(**inputs):
    nsteps = NSTEPS
    if "nc" not in _cached:
        _cached["nc"] = build_program(nsteps)
    nc = _cached["nc"]
    in_maps = make_in_maps(inputs, nsteps)
    res = run_bass_kernel_spmd(nc, in_maps, list(range(R)))
    outs = res.results

    def unshard(name):
        per = [np.asarray(outs[r][name]).transpose(1, 2, 0) for r in range(R)]
        return np.concatenate(per, axis=1)

    return unshard("out_positions"), unshard("out_means"), unshard("out_logvars")
